# revision 7
# baseline (speedup 1.0000x reference)
"""Trainium2 Bass kernel for GumbelNeRF MoE routing.

Math being computed (per point n of N=131072):
  xyz = pe(x[:, :3], 10)  [63]     vd = pe(x[:, 3:6], 4)  [27]
  y   = relu(xyz @ W_enc + b_enc)                     [128]
  S_e = relu(W_sh[e].T @ y + b_sh[e])   e in 0..7     [8,128]
  sig_e = softplus(w_sig . S_e + b_sig)               [8]
  idx = argmax_e( log(sig_e+1e-10)/TEMP + g_e )       (g = fixed gumbel noise)
      = argmax_e( log(sig_e+1e-10) + TEMP*g_e )       (scale-invariant)
  h_e = relu(W_r1[e][:128].T @ S_e + W_r1[e][128:].T @ vd + b_r1[e])  [8,64]
  rgb_e = sigmoid(W_r2[e].T @ h_e + b_r2[e])          [8,3]
  out = [rgb_idx, sig_idx]                            [4]

Strategy: data-parallel over N across 8 cores (weights replicated, no
collectives).  Feature-major layout on device (features on SBUF partitions,
points on the free axis), fp32r matmuls (1 cycle/row at moving dim 512).
Positional encodings and gumbel noise are host-precomputed (input prep only;
they depend on x / constants, not weights).  The straight-through gumbel
softmax reduces to a pure argmax in the forward pass, so no softmax is
computed on device.  rgb expert heads emit point-major (activations
stationary, W_r2 moving), so routing/gating is pointwise per-partition after
tiny PE transposes of the [8, 512] sigma/score tiles.
"""

import os
import numpy as np

N = 131072
E = 8
W = 128
H = 64
NXF = 10
NDF = 4
DX = 3 + 3 * 2 * NXF  # 63
DV = 3 + 3 * 2 * NDF  # 27
TEMP = 0.166667
NCORES = 8
NPC = N // NCORES     # 16384 points per core
NT = 512              # points per tile (PSUM bank limit for f32)

_CACHE = {}


def _build(npc):
    """Build + compile the per-core Bass program for `npc` points."""
    import concourse.bacc as bacc
    import concourse.bass as bass
    import concourse.mybir as mybir
    from concourse import tile

    f32 = mybir.dt.float32
    f32r = mybir.dt.float32r
    AF = mybir.ActivationFunctionType
    ALU = mybir.AluOpType
    AX = mybir.AxisListType
    PSUM = bass.MemorySpace.PSUM

    ntiles = npc // NT
    nc = bacc.Bacc("TRN2", target_bir_lowering=False, debug=False)

    def din(name, shape, dt=f32):
        return nc.dram_tensor(name, shape, dt, kind="ExternalInput").ap()

    xyz_d = din("xyz", [DX, npc], f32r)
    vd_d = din("vd", [DV, npc], f32r)
    g2_d = din("g2", [E, npc])
    wenc_d = din("wenc", [DX, W], f32r)
    wsh_d = din("wsh", [W, E * W], f32r)
    r1s_d = din("r1s", [W, E * H], f32r)
    r1v_d = din("r1v", [DV, 4 * W], f32r)
    wsige_d = din("wsige", [W, E * E], f32r)   # col 8e+e == w_sig, else 0
    r2_d = din("r2", [W, 24], f32r)            # block-diag per pair, col = 6p+3k+c
    benc_d = din("benc", [W, 1])
    bsh_d = din("bsh", [W, E])
    br1_d = din("br1", [W, 4])
    br2_d = din("br2", [1, 24], f32r)
    bsig_d = din("bsig", [E, 1])
    eps_d = din("eps", [E, 1])
    id8_d = din("id8", [E, E])
    ones_d = din("onesw", [1, W], f32r)
    out_d = nc.dram_tensor("out", [npc, 4], f32, kind="ExternalOutput").ap()

    with tile.TileContext(nc) as tc:
        with (
            tc.tile_pool(name="wpool", bufs=1) as wp,
            tc.tile_pool(name="io", bufs=3) as iop,
            tc.tile_pool(name="ypool", bufs=2) as yp,
            tc.tile_pool(name="spool", bufs=4) as sp,
            tc.tile_pool(name="hpool", bufs=2) as hp,
            tc.tile_pool(name="smal", bufs=4) as smp,
            tc.tile_pool(name="outp", bufs=3) as op_,
            tc.tile_pool(name="zpsum", bufs=2, space=PSUM) as zp,
            tc.tile_pool(name="gpsum", bufs=2, space=PSUM) as gp,
            tc.tile_pool(name="rpsum", bufs=2, space=PSUM) as rp,
            tc.tile_pool(name="sgpsum", bufs=1, space=PSUM) as sgp,
            tc.tile_pool(name="tpsum", bufs=1, space=PSUM) as tp,
        ):
            # ---- load weights / consts once ----
            wenc = wp.tile([DX, W], f32r)
            nc.sync.dma_start(wenc[:], wenc_d[:])
            wsh = wp.tile([W, E * W], f32r)
            nc.sync.dma_start(wsh[:], wsh_d[:])
            r1s = wp.tile([W, E * H], f32r)
            nc.sync.dma_start(r1s[:], r1s_d[:])
            r1v = wp.tile([DV, 4 * W], f32r)
            nc.sync.dma_start(r1v[:], r1v_d[:])
            wsige = wp.tile([W, E * E], f32r)
            nc.sync.dma_start(wsige[:], wsige_d[:])
            r2w = wp.tile([W, 24], f32r)
            nc.sync.dma_start(r2w[:], r2_d[:])
            benc = wp.tile([W, 1], f32)
            nc.sync.dma_start(benc[:], benc_d[:])
            bsh = wp.tile([W, E], f32)
            nc.sync.dma_start(bsh[:], bsh_d[:])
            br1 = wp.tile([W, 4], f32)
            nc.sync.dma_start(br1[:], br1_d[:])
            br2 = wp.tile([1, 24], f32r)
            nc.sync.dma_start(br2[:], br2_d[:])
            bsig = wp.tile([E, 1], f32)
            nc.sync.dma_start(bsig[:], bsig_d[:])
            eps = wp.tile([E, 1], f32)
            nc.sync.dma_start(eps[:], eps_d[:])
            id8 = wp.tile([E, E], f32)
            nc.sync.dma_start(id8[:], id8_d[:])
            ones = wp.tile([1, W], f32r)
            nc.sync.dma_start(ones[:], ones_d[:])
            one1 = wp.tile([E, 1], f32)
            nc.vector.memset(one1[:], 1.0)

            for t in range(ntiles):
                sl = slice(t * NT, (t + 1) * NT)

                xyz = iop.tile([DX, NT], f32r, tag="xyz")
                nc.sync.dma_start(xyz[:], xyz_d[:, sl])
                vd = iop.tile([DV, NT], f32r, tag="vd")
                nc.sync.dma_start(vd[:], vd_d[:, sl])
                g2t = iop.tile([E, NT], f32, tag="g2")
                nc.sync.dma_start(g2t[:], g2_d[:, sl])

                # encoder: y = relu(W_enc.T @ xyz + b_enc)
                zy = zp.tile([W, NT], f32, tag="z")
                nc.tensor.matmul(zy[:], wenc[:], xyz[:])
                y = yp.tile([W, NT], f32r)
                nc.scalar.activation(y[:], zy[:], AF.Relu, bias=benc[:])

                psig = sgp.tile([E, NT], f32)     # sigma logits, row/expert
                rgbpm = rp.tile([128, 96], f32)   # point-major rgb logits
                # bias rows for rgb (b_r2 via K=1 ones matmul)
                for j in range(4):
                    nc.tensor.matmul(
                        rgbpm[:, 24 * j:24 * (j + 1)], ones[:], br2[:],
                        start=True, stop=False, skip_group_check=True,
                    )

                for p in range(4):
                    e1, e2 = 2 * p, 2 * p + 1
                    ss = []
                    for e in (e1, e2):
                        ze = zp.tile([W, NT], f32, tag="z")
                        nc.tensor.matmul(
                            ze[:], wsh[:, e * W:(e + 1) * W], y[:]
                        )
                        se = sp.tile([W, NT], f32r, tag="s")
                        nc.scalar.activation(
                            se[:], ze[:], AF.Relu, bias=bsh[:, e:e + 1]
                        )
                        ss.append(se)
                        # sigma logit row e: accumulate w_sig.S_e into psig
                        nc.tensor.matmul(
                            psig[:], wsige[:, E * e:E * (e + 1)], se[:],
                            start=(e == 0), stop=(e == E - 1),
                            skip_group_check=True,
                        )

                    # h pair: PSUM accumulate S-halves then vd across full
                    g1 = gp.tile([W, NT], f32)
                    nc.tensor.matmul(
                        g1[0:H, :], r1s[:, e1 * H:(e1 + 1) * H], ss[0][:],
                        start=True, stop=False, skip_group_check=True,
                    )
                    # fp32r cannot target PSUM base partition 64; use f32
                    nc.tensor.matmul(
                        g1[H:W, :], r1s[:, e2 * H:(e2 + 1) * H].bitcast(f32),
                        ss[1][:].bitcast(f32),
                        start=True, stop=False, skip_group_check=True,
                    )
                    nc.tensor.matmul(
                        g1[:], r1v[:, p * W:(p + 1) * W], vd[:],
                        start=False, stop=True, skip_group_check=True,
                    )
                    hpair = hp.tile([W, NT], f32r)
                    nc.scalar.activation(
                        hpair[:], g1[:], AF.Relu, bias=br1[:, p:p + 1]
                    )
                    # rgb logits, point-major: hpair chunk stationary,
                    # block-diag W_r2 pair moving -> [128pts, 6] at col 24j+6p
                    for j in range(4):
                        cs = slice(j * 128, (j + 1) * 128)
                        nc.tensor.matmul(
                            rgbpm[:, 24 * j + 6 * p:24 * j + 6 * (p + 1)],
                            hpair[:, cs], r2w[:, 6 * p:6 * (p + 1)],
                            start=False, stop=(p == 3), skip_group_check=True,
                        )

                # sigma + score: softplus via ln(exp(z)+1) (one ACT table)
                et = smp.tile([E, NT], f32, tag="et")
                nc.scalar.activation(et[:], psig[:], AF.Exp, bias=bsig[:])
                sig = smp.tile([E, NT], f32, tag="sig")
                nc.scalar.activation(sig[:], et[:], AF.Ln, bias=one1[:])
                v = smp.tile([E, NT], f32, tag="v")
                nc.scalar.activation(v[:], sig[:], AF.Ln, bias=eps[:])
                nc.vector.tensor_tensor(v[:], v[:], g2t[:], ALU.add)

                # sigmoid via 1/(1+exp(-z)); reciprocal on DVE
                rgbs = smp.tile([128, 96], f32, tag="rgbs")
                nc.scalar.activation(rgbs[:], rgbpm[:], AF.Exp, scale=-1.0)
                nc.vector.tensor_scalar(rgbs[:], rgbs[:], 1.0, None, ALU.add)
                nc.vector.reciprocal(rgbs[:], rgbs[:])

                # transpose v/sig to point-major [128, 8] chunks
                tt = tp.tile([128, 64], f32)
                for j in range(4):
                    cs = slice(j * 128, (j + 1) * 128)
                    nc.tensor.transpose(tt[:, 8 * j:8 * j + 8], v[:, cs], id8[:])
                    nc.tensor.transpose(
                        tt[:, 32 + 8 * j:32 + 8 * j + 8], sig[:, cs], id8[:]
                    )

                outpm = op_.tile([128, 16], f32)
                for j in range(4):
                    vT = tt[:, 8 * j:8 * j + 8]
                    sigT = tt[:, 32 + 8 * j:32 + 8 * j + 8]
                    rgbT = rgbs[:, 24 * j:24 * (j + 1)]
                    m = smp.tile([128, 1], f32, tag="m")
                    nc.vector.tensor_reduce(m[:], vT, AX.X, ALU.max)
                    hard = smp.tile([128, E], f32, tag="hard")
                    nc.vector.tensor_scalar(hard[:], vT, m[:], None, ALU.is_equal)
                    spool = smp.tile([128, E], f32, tag="sp")
                    nc.vector.tensor_tensor(spool[:], sigT, hard[:], ALU.mult)
                    nc.vector.tensor_reduce(
                        outpm[:, 4 * j + 3:4 * j + 4], spool[:], AX.X, ALU.add
                    )
                    prod = smp.tile([128, 3, E], f32, tag="prod")
                    nc.vector.tensor_tensor(
                        prod[:],
                        rgbT.rearrange("p (e c) -> p c e", c=3),
                        hard[:].unsqueeze(1).broadcast_to([128, 3, E]),
                        ALU.mult,
                    )
                    nc.vector.tensor_reduce(
                        outpm[:, 4 * j:4 * j + 3], prod[:], AX.X, ALU.add
                    )

                nc.sync.dma_start(
                    out_d[sl].rearrange("(j p) c -> p j c", p=128),
                    outpm[:].rearrange("p (j c) -> p j c", c=4),
                )

    nc.compile()
    return nc


def _host_prep(x, W_enc, b_enc, W_sh, b_sh, w_sig, b_sig, W_r1, b_r1, W_r2, b_r2):
    """Host-side input prep: positional encodings (transposed), gumbel noise,
    weight repacking. All O(MB) numpy work."""
    n = x.shape[0]
    x = np.asarray(x, np.float32)

    def pe_t(xc, deg):
        # feature-major [3 + 6*deg, n]; row order matches reference concat
        out = np.empty((3 + 6 * deg, n), np.float32)
        out[0:3] = xc.T
        for i in range(deg):
            s = np.float32(2.0 ** i)
            out[3 + 3 * i:6 + 3 * i] = np.sin(s * xc).T
            out[3 + 3 * deg + 3 * i:6 + 3 * deg + 3 * i] = np.cos(s * xc).T
        return np.ascontiguousarray(out)

    xyz = pe_t(x[:, :3], NXF)
    vd = pe_t(x[:, 3:6], NDF)

    import jax
    with jax.default_device(jax.devices("cpu")[0]):
        u = np.asarray(
            jax.random.uniform(jax.random.key(42), (n, E), dtype="float32")
        )
    g = -np.log(-np.log(u + np.float32(1e-20)) + np.float32(1e-20))
    g2 = np.ascontiguousarray((np.float32(TEMP) * g).T)  # [E, n]

    W_sh = np.asarray(W_sh, np.float32)
    W_r1 = np.asarray(W_r1, np.float32)
    W_r2 = np.asarray(W_r2, np.float32)
    w_sig = np.asarray(w_sig, np.float32)
    wsh = np.ascontiguousarray(np.transpose(W_sh, (1, 0, 2)).reshape(W, E * W))
    r1s = np.ascontiguousarray(
        np.transpose(W_r1[:, :W, :], (1, 0, 2)).reshape(W, E * H)
    )
    r1v = np.ascontiguousarray(
        np.transpose(W_r1[:, W:, :], (1, 0, 2)).reshape(DV, E * H)
    )
    wsige = np.zeros((W, E * E), np.float32)
    for e in range(E):
        wsige[:, E * e + e] = w_sig
    r2 = np.zeros((W, 24), np.float32)
    for p in range(4):
        r2[0:H, 6 * p:6 * p + 3] = W_r2[2 * p]
        r2[H:W, 6 * p + 3:6 * p + 6] = W_r2[2 * p + 1]
    br1 = np.ascontiguousarray(
        np.asarray(b_r1, np.float32).reshape(4, 2 * H).T
    )  # [128, 4], col p = [b_r1[2p], b_r1[2p+1]]
    feed = {
        "xyz": xyz,
        "vd": vd,
        "g2": g2,
        "wenc": np.ascontiguousarray(np.asarray(W_enc, np.float32)),
        "wsh": wsh,
        "r1s": r1s,
        "r1v": r1v,
        "wsige": wsige,
        "r2": r2,
        "benc": np.asarray(b_enc, np.float32).reshape(W, 1),
        "bsh": np.ascontiguousarray(np.asarray(b_sh, np.float32).T),
        "br1": br1,
        "br2": np.asarray(b_r2, np.float32).reshape(1, 24),
        "bsig": np.full((E, 1), np.float32(np.asarray(b_sig).reshape(-1)[0])),
        "eps": np.full((E, 1), np.float32(1e-10)),
        "id8": np.eye(E, dtype=np.float32),
        "onesw": np.ones((1, W), np.float32),
    }
    return feed


_SHARDED = ("xyz", "vd", "g2")


def kernel(**inputs):
    from concourse.bass_utils import run_bass_kernel_spmd

    feed = _host_prep(**inputs)
    n = feed["xyz"].shape[1]
    npc = n // NCORES

    key = ("nc", npc)
    if key not in _CACHE:
        _CACHE[key] = _build(npc)
    nc = _CACHE[key]

    in_maps = []
    for c in range(NCORES):
        m = {}
        for k, a in feed.items():
            if k in _SHARDED:
                m[k] = np.ascontiguousarray(a[:, c * npc:(c + 1) * npc])
            else:
                m[k] = a
        in_maps.append(m)

    trace = bool(int(os.environ.get("CC_KERNEL_TRACE", "0")))
    res = run_bass_kernel_spmd(
        nc, in_maps, list(range(NCORES)), trace=trace
    )
    _CACHE["last_result"] = res
    out = np.concatenate([res.results[c]["out"] for c in range(NCORES)], axis=0)
    return out
